# revision 13
# baseline (speedup 1.0000x reference)
"""MultiHeadAttn (post-LN, key-padding mask) Trainium2 Bass kernel, 8 cores.

Problem: h [S=2048, B=4, D=1024] f32; 16 heads x 64; key-padding mask [S, B];
out = LayerNorm(h + MHA(h)).

Sharding (head-tensor-parallel): core c handles batch b = c//2 and head-half
hh = c%2 (8 heads), over the FULL 2048-token sequence.  K/V/Q projections are
computed only for the core's own 8 heads (no duplication).  For the output
projection each core emits only its own query half; the partner's 8 heads'
attention vectors for that half arrive via two pairwise AllGather collectives
(issued mid-attention so they hide), and the contraction uses host-built
zero-padded Wo slices so the program stays SPMD-uniform.

Projections run in fp8-e4m3 DoubleRow mode (256-wide contraction per pass,
half the matmuls): host pre-scales Wq/Wk/Wv by 8x to dodge fp8 subnormals;
the 64x score scale folds into the exp scale and the 8x V scale folds into
the softmax-normalize multiply.  kt/qt/v/avt stay bf16.

Per-core device pipeline:
  - warm-up matmuls on a zero tile during the initial DMA wait (HAM clock).
  - attention per (head pair, query half) block, other-half first so the
    exchange pieces ship early: scores^T via row-paired matmuls, exp on
    ScalarE with mask bias, PV + ones-matmul denominators col-paired with
    stationary reuse, reciprocal+normalize on DVE.  Next head pair's K/Q
    fp8 projections interleave as short 8-matmul bursts borrowing scores-
    pool PSUM slots.
  - tail: O-proj over 4 local + 8 gathered e-tiles, residual (bf16,
    prefetched) + LayerNorm fused on DVE/ScalarE.
"""
import numpy as np
import ml_dtypes

N_HEAD, D_MODEL, D_HEAD = 16, 1024, 64
SEQ, BSZ = 2048, 4
SCALE = 1.0 / D_HEAD ** 0.5
W8 = 8.0                    # host pre-scale on Wq/Wk/Wv for fp8 range
LN_EPS = 1e-5
P = 128
NSL = 512                   # matmul moving-operand slab (one PSUM bank fp32)
CC = 4                      # fp8 DoubleRow contraction pair-tiles (1024/256)
EH = 512                    # local e width (8 heads)
ETL = EH // P               # 4 local e-tiles
JT = SEQ // P               # 16 key tiles
JS = SEQ // NSL             # 4 key slabs
OQ = 1024                   # own output queries
TQ = OQ // P                # 8 output row tiles
HPL = ETL                   # 4 local head pairs

_CACHE = {}


def _build():
    from contextlib import ExitStack
    import concourse.bass as bass
    import concourse.mybir as mybir
    import concourse.tile as tile
    from concourse import bacc

    dt = mybir.dt
    f32, bf16, f8 = dt.float32, dt.bfloat16, dt.float8e4
    AF = mybir.ActivationFunctionType
    ALU = mybir.AluOpType
    DR = mybir.MatmulPerfMode.DoubleRow

    nc = bacc.Bacc(None, target_bir_lowering=False)

    hT8 = nc.dram_tensor("hT8", [D_MODEL, SEQ], f8, kind="ExternalInput")
    hq = nc.dram_tensor("hq", [OQ, D_MODEL], bf16, kind="ExternalInput")
    wq8 = nc.dram_tensor("wq8", [D_MODEL, EH], f8, kind="ExternalInput")
    wk8 = nc.dram_tensor("wk8", [D_MODEL, EH], f8, kind="ExternalInput")
    wv8 = nc.dram_tensor("wv8", [D_MODEL, EH], f8, kind="ExternalInput")
    wo_own = nc.dram_tensor("wo_own", [EH, D_MODEL], bf16, kind="ExternalInput")
    wo_g1 = nc.dram_tensor("wo_g1", [EH, D_MODEL], bf16, kind="ExternalInput")
    wo_g2 = nc.dram_tensor("wo_g2", [EH, D_MODEL], bf16, kind="ExternalInput")
    mb = nc.dram_tensor("mb", [SEQ], f32, kind="ExternalInput")
    gam = nc.dram_tensor("gam", [D_MODEL], f32, kind="ExternalInput")
    bet = nc.dram_tensor("bet", [D_MODEL], f32, kind="ExternalInput")
    out = nc.dram_tensor("out", [OQ, D_MODEL], f32, kind="ExternalOutput")

    groups = [[0, 1], [2, 3], [4, 5], [6, 7]]

    with tile.TileContext(nc) as tc, ExitStack() as ctx:
        persist = ctx.enter_context(tc.tile_pool(name="persist", bufs=1))

        kt_sb = [persist.tile([P, SEQ], bf16, name=f"kt{e}") for e in range(ETL)]
        qt_sb = [persist.tile([P, SEQ], bf16, name=f"qt{e}") for e in range(ETL)]
        v_sb = [persist.tile([P, 8, D_HEAD], bf16, name=f"v{t}")
                for t in range(JT)]
        avt_sb = [persist.tile([P, SEQ], bf16, name=f"avt{e}") for e in range(ETL)]
        ones64 = persist.tile([P, 64], bf16, name="ones64")
        mask_sb = persist.tile([P, JT], f32, name="mask")
        eps_sb = persist.tile([P, 1], f32, name="eps")
        warm_a = persist.tile([P, P], bf16, name="warma")
        warm_b = persist.tile([P, NSL], bf16, name="warmb")

        nc.vector.memset(eps_sb, LN_EPS)
        nc.vector.memset(ones64, 1.0)
        nc.vector.memset(warm_a, 0.0)
        nc.vector.memset(warm_b, 0.0)

        nc.gpsimd.dma_start(out=mask_sb,
                            in_=bass.AP(tensor=mb, offset=0, ap=[[1, P], [P, JT]]))

        # ---- tail weights/data: loaded during attention ---------------------
        w3p = ctx.enter_context(tc.tile_pool(name="w3p", bufs=1))
        woo_sb = [w3p.tile([P, D_MODEL], bf16, name=f"woo{c}") for c in range(ETL)]
        wg1_sb = [w3p.tile([P, D_MODEL], bf16, name=f"wg1{c}") for c in range(ETL)]
        wg2_sb = [w3p.tile([P, D_MODEL], bf16, name=f"wg2{c}") for c in range(ETL)]
        gam_sb = w3p.tile([P, D_MODEL], f32, name="gamr")
        bet_sb = w3p.tile([P, D_MODEL], f32, name="betr")
        hq_sb = [w3p.tile([P, D_MODEL], bf16, name=f"hq{t}") for t in range(TQ)]
        rx_sb = [w3p.tile([P, OQ], bf16, name=f"rx{i}") for i in range(2 * ETL)]

        # ---- DRAM bounce buffers for the avt exchange -----------------------
        dramp = ctx.enter_context(tc.tile_pool(name="dramp", bufs=1, space="DRAM"))
        cc_in1 = dramp.tile([2 * P, OQ], bf16, name="ccin1")
        cc_out1 = dramp.tile([4 * P, OQ], bf16, name="ccout1")
        cc_in2 = dramp.tile([2 * P, OQ], bf16, name="ccin2")
        cc_out2 = dramp.tile([4 * P, OQ], bf16, name="ccout2")

        # ---- phase 1 scope: fp8 h^T pair-tiles + streamed W columns ---------
        ph1_ctx = ExitStack()
        ph1 = ph1_ctx.enter_context(tc.tile_pool(name="ph1", bufs=1))
        ht8_sb = [ph1.tile([P, 2, SEQ], f8, name=f"ht{c}") for c in range(CC)]
        wv8_sb = [ph1.tile([P, 2, EH], f8, name=f"wv{c}") for c in range(CC)]

        wcol = ph1_ctx.enter_context(tc.tile_pool(name="wcol", bufs=3))

        def load_wcol(w, e, tag):
            wc = wcol.tile([P, CC, 2, P], f8, tag=tag, name=f"{tag}{e}")
            nc.sync.dma_start(
                out=wc,
                in_=w[:, e * P:(e + 1) * P].rearrange(
                    "(cc k p) e -> p cc k e", p=P, k=2))
            return wc

        # startup DMA priority: wk col 0 first, then fp8 h^T pair tiles.
        wc0 = load_wcol(wk8, 0, "wkc")
        for c in range(CC):
            eng = nc.sync if c % 2 == 0 else nc.scalar
            eng.dma_start(
                out=ht8_sb[c],
                in_=hT8[2 * c * P:(2 * c + 2) * P, :].rearrange(
                    "(k p) t -> p k t", p=P))
            eng.dma_start(
                out=wv8_sb[c],
                in_=wv8[2 * c * P:(2 * c + 2) * P, :].rearrange(
                    "(k p) e -> p k e", p=P))

        pre_ctx = ExitStack()
        psA = pre_ctx.enter_context(tc.tile_pool(name="psA", bufs=6, space="PSUM"))

        # PE warm-up: keep the HAM activity window busy while input DMAs land.
        for k in range(24):
            ps = psA.tile([P, NSL], f32, tag="psa", name=f"warm{k}")
            nc.tensor.matmul(ps, warm_a, warm_b, start=True, stop=True)

        def kq_etile(wc, dst):
            """Project one K/Q e-tile (fp8 DR): cc-outer, 4 moving slabs."""
            pss = [psA.tile([P, NSL], f32, tag="psa", name=f"pp{id(wc)}_{j}")
                   for j in range(JS)]
            for c in range(CC):
                for j in range(JS):
                    nc.tensor.matmul(pss[j], wc[:, c, :, :],
                                     ht8_sb[c][:, :, j * NSL:(j + 1) * NSL],
                                     start=(c == 0), stop=(c == CC - 1),
                                     perf_mode=DR)
            for j in range(JS):
                nc.vector.tensor_copy(dst[:, j * NSL:(j + 1) * NSL], pss[j])

        kq_etile(wc0, kt_sb[0])
        wcq0 = load_wcol(wq8, 0, "wqc")
        kq_etile(wcq0, qt_sb[0])

        # V projection (fp8 DR): stationary h^T pair key-tiles, moving Wv
        for t in range(JT):
            ps = psA.tile([P, NSL], f32, tag="psa", name=f"psv{t}")
            for c in range(CC):
                nc.tensor.matmul(ps, ht8_sb[c][:, :, t * P:(t + 1) * P],
                                 wv8_sb[c], start=(c == 0), stop=(c == CC - 1),
                                 perf_mode=DR)
            nc.vector.tensor_copy(
                v_sb[t], ps[:, :].rearrange("p (h d) -> p h d", d=D_HEAD))
        pre_ctx.close()

        # stage all remaining K/Q weight columns now (sync queue, ahead of any
        # collective readback so no head-of-line stall mid-attention)
        wc_all = {("k", 0): wc0, ("q", 0): wcq0}
        for e in range(1, HPL):
            wc_all[("k", e)] = load_wcol(wk8, e, "wkc")
            wc_all[("q", e)] = load_wcol(wq8, e, "wqc")

        # tail-weight DMAs (run while attention computes)
        for c in range(ETL):
            nc.sync.dma_start(out=woo_sb[c], in_=wo_own[c * P:(c + 1) * P, :])
            nc.sync.dma_start(out=wg1_sb[c], in_=wo_g1[c * P:(c + 1) * P, :])
            nc.sync.dma_start(out=wg2_sb[c], in_=wo_g2[c * P:(c + 1) * P, :])
        nc.gpsimd.dma_start(out=gam_sb,
                            in_=bass.AP(tensor=gam, offset=0, ap=[[0, P], [1, D_MODEL]]))
        nc.gpsimd.dma_start(out=bet_sb,
                            in_=bass.AP(tensor=bet, offset=0, ap=[[0, P], [1, D_MODEL]]))

        # ---- attention ------------------------------------------------------
        attn_ctx = ExitStack()
        scp = attn_ctx.enter_context(tc.tile_pool(name="scp", bufs=2, space="PSUM"))
        avp = attn_ctx.enter_context(tc.tile_pool(name="avp", bufs=2, space="PSUM"))
        ptp = attn_ctx.enter_context(tc.tile_pool(name="ptp", bufs=8))
        nrm = attn_ctx.enter_context(tc.tile_pool(name="nrm", bufs=3))

        IS = OQ // NSL  # 2 query slabs per block

        def emit_pv(av, den, hp, j, pts):
            first, last = (j == 0), (j == JT - 1)
            for hb in range(2):
                base = hb * 64
                for i in range(IS):
                    nc.tensor.matmul(av[i][base:base + 64, :],
                                     v_sb[j][:, hp * 2 + hb, :],
                                     pts[hb][:, i * NSL:(i + 1) * NSL],
                                     start=first, stop=last,
                                     tile_position=(0, base),
                                     skip_group_check=(hb == 1))
            for hb in range(2):
                base = hb * 64
                for i in range(IS):
                    nc.tensor.matmul(den[i][base:base + 64, :], ones64,
                                     pts[hb][:, i * NSL:(i + 1) * NSL],
                                     start=first, stop=last,
                                     tile_position=(0, base),
                                     skip_group_check=True)

        for blk in range(2 * HPL):
            hp, qh = blk // 2, 1 - blk % 2   # other-half (qh=1) first
            q0 = qh * OQ
            av = [avp.tile([P, NSL], f32, tag="av", name=f"av{blk}_{i}")
                  for i in range(IS)]
            den = [avp.tile([P, NSL], f32, tag="den", name=f"den{blk}_{i}")
                   for i in range(IS)]
            # interleaved next-head-pair fp8 projections: 4-MM single-slab
            # bursts so the borrowed scores-pool slot is held no longer than
            # a regular scores tile (keeps the exp stream fed); K in the
            # first block of the pair, Q in the second.
            if hp + 1 < HPL:
                kind = "k" if qh == 1 else "q"
                proj_work = {3: (kind, 0), 6: (kind, 1), 9: (kind, 2),
                             12: (kind, 3)}
            else:
                proj_work = {}
            prev_pt = None

            for j in range(JT):
                cur_pt = []
                for hb in range(2):
                    base = hb * 64
                    sc = scp.tile([P, OQ], f32, tag="sc", name=f"sc{blk}_{j}_{hb}")
                    for i in range(IS):
                        nc.tensor.matmul(
                            sc[:, i * NSL:(i + 1) * NSL],
                            kt_sb[hp][base:base + 64, j * P:(j + 1) * P],
                            qt_sb[hp][base:base + 64, q0 + i * NSL:q0 + (i + 1) * NSL],
                            start=True, stop=True, tile_position=(base, 0))
                    pt_t = ptp.tile([P, OQ], bf16, tag="pt",
                                    name=f"pt{blk}_{j}_{hb}")
                    nc.scalar.activation(pt_t, sc, AF.Exp,
                                         bias=mask_sb[:, j:j + 1],
                                         scale=SCALE / (W8 * W8))
                    cur_pt.append(pt_t)

                if prev_pt is not None:
                    emit_pv(av, den, hp, j - 1, prev_pt)
                prev_pt = cur_pt

                if j in proj_work:
                    kind, sl = proj_work[j]
                    borrow = scp.tile([P, OQ], f32, tag="sc", name=f"bw{blk}_{j}")
                    wc = wc_all[(kind, hp + 1)]
                    dst = kt_sb[hp + 1] if kind == "k" else qt_sb[hp + 1]
                    for c in range(CC):
                        nc.tensor.matmul(borrow[:, 0:NSL],
                                         wc[:, c, :, :],
                                         ht8_sb[c][:, :, sl * NSL:(sl + 1) * NSL],
                                         start=(c == 0), stop=(c == CC - 1),
                                         perf_mode=DR)
                    nc.vector.tensor_copy(
                        dst[:, sl * NSL:(sl + 1) * NSL], borrow[:, 0:NSL])

            emit_pv(av, den, hp, JT - 1, prev_pt)

            # normalize: evacuate psum, reciprocal, scale into avt (undo W8)
            for i in range(IS):
                avc = nrm.tile([P, NSL], f32, tag="avc", name=f"avc{blk}_{i}")
                nc.vector.tensor_copy(avc, av[i])
                dnc = nrm.tile([P, NSL], f32, tag="dnc", name=f"dnc{blk}_{i}")
                nc.vector.tensor_copy(dnc, den[i])
                rep = nrm.tile([P, NSL], f32, tag="rep", name=f"rep{blk}_{i}")
                nc.vector.reciprocal(rep, dnc)
                for hb in range(2):
                    nc.vector.scalar_tensor_tensor(
                        out=avt_sb[hp][hb * 64:(hb + 1) * 64,
                                       q0 + i * NSL:q0 + (i + 1) * NSL],
                        in0=avc[hb * 64:(hb + 1) * 64, :], scalar=1.0 / W8,
                        in1=rep[hb * 64:(hb + 1) * 64, :],
                        op0=ALU.mult, op1=ALU.mult)

            # exchange: the other-half block runs first per head pair, so its
            # piece ships early; collectives sit fully under attention.
            if qh == 1:
                cc_in = cc_in1 if hp < 2 else cc_in2
                r0 = (hp % 2) * P
                nc.sync.dma_start(out=cc_in[r0:r0 + P, :],
                                  in_=avt_sb[hp][:, OQ:])
            if blk == 2:
                nc.gpsimd.collective_compute(
                    "AllGather", mybir.AluOpType.bypass,
                    replica_groups=groups,
                    ins=[cc_in1[:].opt()], outs=[cc_out1[:].opt()])
                for i in range(4):
                    nc.gpsimd.dma_start(out=rx_sb[i],
                                        in_=cc_out1[i * P:(i + 1) * P, :])
            if blk == 6:
                nc.gpsimd.collective_compute(
                    "AllGather", mybir.AluOpType.bypass,
                    replica_groups=groups,
                    ins=[cc_in2[:].opt()], outs=[cc_out2[:].opt()])
                for i in range(4):
                    nc.gpsimd.dma_start(out=rx_sb[4 + i],
                                        in_=cc_out2[i * P:(i + 1) * P, :])
            # prefetch residual rows late (after h^T pressure drops)
            if blk == 5:
                for t in range(TQ):
                    nc.sync.dma_start(out=hq_sb[t], in_=hq[t * P:(t + 1) * P, :])

        # ---- output projection + residual + layernorm -----------------------
        attn_ctx.close()
        ph1_ctx.close()

        pso = ctx.enter_context(tc.tile_pool(name="pso", bufs=6, space="PSUM"))
        lnp = ctx.enter_context(tc.tile_pool(name="lnp", bufs=3))
        lns = ctx.enter_context(tc.tile_pool(name="lns", bufs=8))

        osrc = ([(avt_sb[e], woo_sb[e]) for e in range(ETL)]
                + [(rx_sb[i], wg1_sb[i]) for i in range(ETL)]
                + [(rx_sb[4 + i], wg2_sb[i]) for i in range(ETL)])

        for t in range(TQ):
            xs = lnp.tile([P, D_MODEL], f32, tag="xs", name=f"xs{t}")
            sums = lns.tile([P, 2], f32, tag="sm", name=f"sm{t}")
            pss = [pso.tile([P, NSL], f32, tag="po", name=f"po{t}_{m}")
                   for m in range(2)]
            ne = len(osrc)
            for e, (src, wt) in enumerate(osrc):
                st = src[:, t * P:(t + 1) * P]
                for m in range(2):
                    nc.tensor.matmul(pss[m], st, wt[:, m * NSL:(m + 1) * NSL],
                                     start=(e == 0), stop=(e == ne - 1))
            for m in range(2):
                nc.vector.scalar_tensor_tensor(
                    out=xs[:, m * NSL:(m + 1) * NSL], in0=pss[m], scalar=1.0,
                    in1=hq_sb[t][:, m * NSL:(m + 1) * NSL],
                    op0=ALU.mult, op1=ALU.add,
                    accum_out=sums[:, m:m + 1])
            sq = lns.tile([P, 2], f32, tag="sq", name=f"sq{t}")
            xsq = lnp.tile([P, D_MODEL], f32, tag="xq", name=f"xq{t}")
            for m in range(2):
                nc.scalar.activation(xsq[:, m * NSL:(m + 1) * NSL],
                                     xs[:, m * NSL:(m + 1) * NSL], AF.Square,
                                     accum_out=sq[:, m:m + 1])
            mean = lns.tile([P, 1], f32, tag="mn", name=f"mn{t}")
            nc.vector.tensor_add(mean, sums[:, 0:1], sums[:, 1:2])
            nc.vector.tensor_scalar_mul(mean, mean, 1.0 / D_MODEL)
            msq = lns.tile([P, 1], f32, tag="mq", name=f"mq{t}")
            nc.vector.tensor_mul(msq, mean, mean)
            var = lns.tile([P, 1], f32, tag="vr", name=f"vr{t}")
            nc.vector.tensor_add(var, sq[:, 0:1], sq[:, 1:2])
            nc.vector.scalar_tensor_tensor(
                out=var, in0=var, scalar=1.0 / D_MODEL, in1=msq,
                op0=ALU.mult, op1=ALU.subtract)
            std = lns.tile([P, 1], f32, tag="sd", name=f"sd{t}")
            nc.scalar.activation(std, var, AF.Sqrt, bias=eps_sb[:, 0:1])
            rstd = lns.tile([P, 1], f32, tag="rs", name=f"rs{t}")
            nc.vector.reciprocal(rstd, std)
            nmr = lns.tile([P, 1], f32, tag="nm", name=f"nm{t}")
            nc.vector.tensor_scalar_mul(nmr, mean, -1.0)
            gs = lnp.tile([P, D_MODEL], f32, tag="gs", name=f"gs{t}")
            nc.vector.tensor_scalar(out=gs, in0=gam_sb,
                                    scalar1=rstd[:, 0:1], scalar2=None,
                                    op0=ALU.mult)
            xg = lnp.tile([P, D_MODEL], f32, tag="xg", name=f"xg{t}")
            nc.vector.scalar_tensor_tensor(
                out=xg, in0=xs, scalar=nmr[:, 0:1], in1=gs,
                op0=ALU.add, op1=ALU.mult)
            xn = lnp.tile([P, D_MODEL], f32, tag="xn", name=f"xn{t}")
            if t % 2 == 0:
                nc.gpsimd.tensor_add(xn, xg, bet_sb)
            else:
                nc.vector.tensor_add(xn, xg, bet_sb)
            nc.sync.dma_start(out=out[t * P:(t + 1) * P, :], in_=xn)

    nc.compile()
    return nc


def _get_nc():
    if "nc" not in _CACHE:
        _CACHE["nc"] = _build()
    return _CACHE["nc"]


def _make_in_maps(inputs):
    bf = ml_dtypes.bfloat16
    f8 = ml_dtypes.float8_e4m3
    h = np.asarray(inputs["h"], dtype=np.float32)
    mask = np.asarray(inputs["attn_mask"])
    Wq = np.asarray(inputs["Wq"], dtype=np.float32)
    Wkv = np.asarray(inputs["Wkv"], dtype=np.float32)
    Wo = np.asarray(inputs["Wo"], dtype=np.float32)
    gamma = np.asarray(inputs["gamma"], dtype=np.float32)
    beta = np.asarray(inputs["beta"], dtype=np.float32)

    Wk = Wkv[:, :D_MODEL]
    Wv = Wkv[:, D_MODEL:]
    wo_b = Wo.astype(bf)
    z256 = np.zeros((256, D_MODEL), dtype=bf)

    in_maps = []
    for c in range(8):
        b, hh = divmod(c, 2)
        hb = h[:, b, :]
        own = slice(hh * OQ, (hh + 1) * OQ)
        other = slice((1 - hh) * OQ, (2 - hh) * OQ)
        hT_b = hb.T
        hT_r = np.ascontiguousarray(np.concatenate(
            [hT_b[:, own], hT_b[:, other]], axis=1).astype(f8))
        mb_full = np.where(mask[:, b], np.float32(-1e9), np.float32(0.0))
        heads = slice(hh * EH, (hh + 1) * EH)
        # AllGather row layouts: cc_out1 = [heads 0-3 | heads 8-11],
        # cc_out2 = [heads 4-7 | heads 12-15]; zero my own echoed half.
        if hh == 0:
            wg1 = np.concatenate([z256, wo_b[512:768]])
            wg2 = np.concatenate([z256, wo_b[768:1024]])
        else:
            wg1 = np.concatenate([wo_b[0:256], z256])
            wg2 = np.concatenate([wo_b[256:512], z256])
        in_maps.append({
            "hT8": hT_r,
            "hq": np.ascontiguousarray(hb[own, :].astype(bf)),
            "wq8": np.ascontiguousarray((Wq[:, heads] * W8).astype(f8)),
            "wk8": np.ascontiguousarray((Wk[:, heads] * W8).astype(f8)),
            "wv8": np.ascontiguousarray((Wv[:, heads] * W8).astype(f8)),
            "wo_own": np.ascontiguousarray(wo_b[heads, :]),
            "wo_g1": np.ascontiguousarray(wg1),
            "wo_g2": np.ascontiguousarray(wg2),
            "mb": np.ascontiguousarray(
                np.concatenate([mb_full[own], mb_full[other]])),
            "gam": gamma, "bet": beta,
        })
    return in_maps


def _run(in_maps, **kwargs):
    from concourse.bass_utils import run_bass_kernel_spmd
    return run_bass_kernel_spmd(_get_nc(), in_maps, core_ids=list(range(8)),
                                **kwargs)


def kernel(**inputs) -> np.ndarray:
    res = _run(_make_in_maps(inputs))
    out = np.empty((SEQ, BSZ, D_MODEL), dtype=np.float32)
    for c in range(8):
        b, hh = divmod(c, 2)
        out[hh * OQ:(hh + 1) * OQ, :, :][:, b, :] = res.results[c]["out"]
    return out
